# revision 24
# baseline (speedup 1.0000x reference)
"""Trainium2 Bass kernel for nn_ConvEmbeddingXY (retrieval_knn).

Problem: B=32 batches of N=1000 2-D points. Per point: node embedding
(x @ W1 + b1), 10-NN by squared distance (incl. self), neighbor coords
sorted by x and by y feed two tiny convs, conv outputs go through W2 and
sum with the node embedding.

Strategy (data-parallel over B across 8 cores, 4 batches/core), v2:
  - distances via a split-precision bf16 PE matmul (x = xhi + xlo in bf16;
    u = 2x_i.x_j - r_j up to ~1e-5 abs error; the per-row -r_i shift is
    dropped entirely since per-row ranking is shift-invariant)
  - exact top-16 candidates via DVE max8/max_index/match_replace on the
    f32 psum copy of u
  - candidate (x,y) pairs via GPSIMD indirect_copy (16-partition-group
    shared index stream) + masked pick; the pick multiply runs on GPSIMD,
    the pick reduce on DVE
  - refine: d2 recomputed exactly like the reference from gathered f32
    coords; top-10 marked via a +1e30-sentinel match_replace round and a
    per-partition threshold compare against the 10th value
  - per-axis sort of the 10 pairs via max8 on negated coords; companions
    via a one-hot (GPSIMD is_equal+multiply) and one fused DVE reduce
  - output: fp16 F[43] features transposed on PE, one fp16 matmul with the
    host-folded [43, H] weight matrix
  - the per-chunk work is software-pipelined at depth 3 (A: matmul+scan+
    gather, B: pick+refine+sort, C: companions+output) so the DVE never
    waits on GPSIMD round-trips.
"""

import numpy as np

B, N, K, H, C = 32, 1000, 10, 128, 2
NPAD = 1024
NCORES = 8
BL = B // NCORES          # batches per core
NCHUNK = NPAD // 128      # 128-point chunks per batch
NF = 43                   # x,y | xvals 10 | yvals 10 | xcomps 10 | ycomps 10 | 1

_SENT = 1.0e30            # refine round-1 sentinel (positive!)


def _split_multiwaits(nc, mybir):
    """This container's walrus build accepts at most ONE sync-wait command per
    instruction. Tile attaches several; redistribute extras onto same-engine
    NoOp carriers placed immediately before the instruction."""
    counter = 0
    for fn in nc.m.functions:
        for blk in fn.blocks:
            insts = blk.instructions
            new = []
            changed = False
            for inst in insts:
                si = inst.sync_info
                waits = list(si.on_wait) if (si is not None and si.on_wait) else []
                if len(waits) > 1:
                    for w in waits[:-1]:
                        counter += 1
                        nop = mybir.InstNoOp(
                            name=f"I-waitcarrier-{counter}", ins=[], outs=[]
                        )
                        nop.engine = inst.engine
                        nop.sync_info = mybir.SyncInfo(on_wait=[w], on_update=[])
                        new.append(nop)
                    inst.sync_info = mybir.SyncInfo(
                        on_wait=[waits[-1]],
                        on_update=list(si.on_update) if si.on_update else [],
                    )
                    changed = True
                new.append(inst)
            if changed:
                blk.instructions = new


def _build_program(debug=False, split=True):
    import concourse.bass as bass
    import concourse.mybir as mybir
    from concourse.tile import TileContext

    f32 = mybir.dt.float32
    f16 = mybir.dt.float16
    bf16 = mybir.dt.bfloat16
    u16 = mybir.dt.uint16
    u8 = mybir.dt.uint8
    AF = mybir.ActivationFunctionType
    OP = mybir.AluOpType

    nc = bass.Bass()

    lhsd = nc.dram_tensor("lhsd", [BL, 8, NPAD], bf16, kind="ExternalInput")
    rhsd = nc.dram_tensor("rhsd", [BL, 8, NPAD], bf16, kind="ExternalInput")
    xytabd = nc.dram_tensor("xytabd", [BL, 128, 2 * NPAD], f32, kind="ExternalInput")
    auxd = nc.dram_tensor("auxd", [BL, 128, 40], f32, kind="ExternalInput")
    pmask_d = nc.dram_tensor("pmask", [128, 16], f32, kind="ExternalInput")
    ident_d = nc.dram_tensor("ident", [128, 128], f16, kind="ExternalInput")
    mt_d = nc.dram_tensor("mt", [NF, 128], f16, kind="ExternalInput")

    y = nc.dram_tensor("y", [BL, N, H], f32, kind="ExternalOutput")
    if debug:
        d_idx = nc.dram_tensor("d_idx", [BL, NCHUNK, 128, 16], u16, kind="ExternalOutput")
        d_cxy = nc.dram_tensor("d_cxy", [BL, NCHUNK, 128, 32], f32, kind="ExternalOutput")
        d_mask = nc.dram_tensor("d_mask", [BL, NCHUNK, 128, 32], u8, kind="ExternalOutput")
        d_f = nc.dram_tensor("d_f", [BL, NCHUNK, 128, NF], f16, kind="ExternalOutput")

    with TileContext(nc) as tc:
        with (
            tc.tile_pool(name="const", bufs=1) as constp,
            tc.tile_pool(name="batch", bufs=2) as batchp,
            tc.tile_pool(name="big", bufs=3) as bigp,
            tc.tile_pool(name="small", bufs=4) as smallp,
            tc.tile_pool(name="psumG", bufs=2, space="PSUM") as psumGp,
            tc.tile_pool(name="psumT", bufs=2, space="PSUM") as psumTp,
            tc.tile_pool(name="psumO", bufs=2, space="PSUM") as psumOp,
        ):
            pmask = constp.tile([128, 16], f32)
            ident = constp.tile([128, 128], f16)
            mt = constp.tile([NF, 128], f16)
            nc.sync.dma_start(pmask[:], pmask_d[:])
            nc.sync.dma_start(ident[:], ident_d[:])
            nc.sync.dma_start(mt[:], mt_d[:])

            batch_tiles = {}

            def load_batch(b):
                lhs_sb = batchp.tile([8, NPAD], bf16, tag="lhs")
                rhs_sb = batchp.tile([8, NPAD], bf16, tag="rhs")
                xytab = batchp.tile([128, 2 * NPAD], f32, tag="xytab")
                aux = batchp.tile([128, 40], f32, tag="aux")
                nc.sync.dma_start(lhs_sb[:], lhsd[b])
                nc.sync.dma_start(rhs_sb[:], rhsd[b])
                nc.sync.dma_start(xytab[:], xytabd[b])
                nc.sync.dma_start(aux[:], auxd[b])
                batch_tiles[b] = (lhs_sb, rhs_sb, xytab, aux)

            state = {}

            def stage_a(b, t):
                if t == 0 and b not in batch_tiles:
                    load_batch(b)
                lhs_sb, rhs_sb, xytab, aux = batch_tiles[b]
                n0 = 128 * t

                psum_g = psumGp.tile([128, N], f32, tag="g")
                lhsT = lhs_sb[:, n0 : n0 + 128]
                nc.tensor.matmul(psum_g[:, 0:512], lhsT, rhs_sb[:, 0:512],
                                 start=True, stop=True)
                nc.tensor.matmul(psum_g[:, 512:N], lhsT, rhs_sb[:, 512:N],
                                 start=True, stop=True)

                negr = aux[:, t : t + 1]
                u = bigp.tile([128, N], f32, tag="u")
                nc.scalar.activation(u[:, 0:512], psum_g[:, 0:512], AF.Identity,
                                     bias=negr, scale=1.0)
                nc.scalar.activation(u[:, 512:N], psum_g[:, 512:N], AF.Identity,
                                     bias=negr, scale=1.0)

                m8a = smallp.tile([128, 8], f32, tag="m8a")
                m8b = smallp.tile([128, 8], f32, tag="m8b")
                idx16 = smallp.tile([128, 16], u16, tag="idx16")
                idx2 = smallp.tile([128, 16], u16, tag="idx2")
                gath = bigp.tile([128, 512], f32, tag="gath")
                data3 = xytab[:].rearrange("p (n c) -> p n c", n=NPAD, c=2)
                nc.vector.max(out=m8a[:], in_=u[:])
                nc.vector.max_index(out=idx16[:, 0:8], in_max=m8a[:], in_values=u[:])
                nc.vector.tensor_scalar(out=idx2[:, 0:8], in0=idx16[:, 0:8],
                                        scalar1=2, scalar2=None, op0=OP.mult)
                nc.gpsimd.indirect_copy(
                    out=gath[:, 0:256].rearrange("p (i c) -> p i c", i=128, c=2),
                    data=data3, idxs=idx2[:, 0:8],
                    i_know_ap_gather_is_preferred=True,
                )
                nc.vector.match_replace(out=u[:], in_to_replace=m8a[:],
                                        in_values=u[:], imm_value=-1.0e30)
                nc.vector.max(out=m8b[:], in_=u[:])
                nc.vector.max_index(out=idx16[:, 8:16], in_max=m8b[:], in_values=u[:])
                nc.vector.tensor_scalar(out=idx2[:, 8:16], in0=idx16[:, 8:16],
                                        scalar1=2, scalar2=None, op0=OP.mult)
                nc.gpsimd.indirect_copy(
                    out=gath[:, 256:512].rearrange("p (i c) -> p i c", i=128, c=2),
                    data=data3, idxs=idx2[:, 8:16],
                    i_know_ap_gather_is_preferred=True,
                )
                state[(b, t)] = (gath, idx16)

            def stage_b(b, t):
                _, _, _, aux = batch_tiles[b]
                gath, idx16 = state[(b, t)]

                tmp = bigp.tile([128, 512], f32, tag="gtmp")
                gv = gath[:].rearrange("p (m s c) -> p m s c", m=16, s=16, c=2)
                pm = pmask[:].unsqueeze(1).unsqueeze(3).to_broadcast([128, 16, 16, 2])
                nc.vector.tensor_tensor(
                    out=tmp[:].rearrange("p (m s c) -> p m s c", m=16, s=16, c=2),
                    in0=gv, in1=pm, op=OP.mult)
                # pick: cxy[p, 16c+m] = sum_s tmp[p, m, s, c]
                cxy = smallp.tile([128, 32], f32, tag="cxy")
                nc.vector.tensor_reduce(
                    out=cxy[:].rearrange("p (c m) -> p m c", c=2, m=16),
                    in_=tmp[:].rearrange("p (m s c) -> p m s c", m=16, s=16, c=2)
                        .transpose([0, 1, 3, 2]),
                    axis=mybir.AxisListType.X, op=OP.add)
                cx16 = cxy[:, 0:16]
                cy16 = cxy[:, 16:32]

                # refine: d2 recomputed exactly as in the reference
                negx = aux[:, 8 + t : 9 + t]
                negy = aux[:, 16 + t : 17 + t]
                dx2 = smallp.tile([128, 16], f32, tag="dx2")
                dy2 = smallp.tile([128, 16], f32, tag="dy2")
                nc.scalar.activation(dx2[:], cx16, AF.Square, bias=negx, scale=1.0)
                nc.scalar.activation(dy2[:], cy16, AF.Square, bias=negy, scale=1.0)
                s16 = smallp.tile([128, 16], f32, tag="s16")
                nc.vector.scalar_tensor_tensor(out=s16[:], in0=dx2[:], scalar=-1.0,
                                               in1=dy2[:], op0=OP.mult,
                                               op1=OP.subtract)
                # find v10 = 10th-largest of s16 on a scratch copy
                s16b = smallp.tile([128, 16], f32, tag="s16b")
                nc.vector.tensor_copy(out=s16b[:], in_=s16[:])
                mc1 = smallp.tile([128, 8], f32, tag="mc1")
                nc.vector.max(out=mc1[:], in_=s16b[:])
                nc.vector.match_replace(out=s16b[:], in_to_replace=mc1[:],
                                        in_values=s16b[:], imm_value=-_SENT)
                mc2 = smallp.tile([128, 8], f32, tag="mc2")
                nc.vector.max(out=mc2[:], in_=s16b[:])
                # included = s16 >= v10 (exactly the top-10 slots, ties aside)
                maskin = smallp.tile([128, 16], u8, tag="maskin")
                nc.vector.tensor_scalar(out=maskin[:], in0=s16[:],
                                        scalar1=mc2[:, 1:2], scalar2=None,
                                        op0=OP.is_ge)

                # keys: kx = -4 default, included slots -> +coord (sort DESC;
                # the host reverses the MT rows to undo the ordering)
                kx = smallp.tile([128, 32], f32, tag="kx")
                nc.vector.memset(kx[:], -4.0)
                nc.vector.copy_predicated(
                    out=kx[:].rearrange("p (a j) -> p a j", a=2, j=16),
                    mask=maskin[:].unsqueeze(1).to_broadcast([128, 2, 16]),
                    data=cxy[:].rearrange("p (a j) -> p a j", a=2, j=16))

                F = smallp.tile([128, NF], f16, tag="F")
                nc.scalar.activation(F[:, 0:2], aux[:, 24 + 2 * t : 26 + 2 * t],
                                     AF.Copy, bias=0.0, scale=1.0)
                nc.vector.memset(F[:, 42:43], 1.0)

                ordb = smallp.tile([128, 32], u16, tag="ordb")
                for a in (0, 1):
                    ka = kx[:, 16 * a : 16 * a + 16]
                    mk1 = smallp.tile([128, 8], f32, tag=f"mk1{a}")
                    nc.vector.max(out=mk1[:], in_=ka)
                    nc.vector.max_index(out=ordb[:, 16 * a : 16 * a + 8],
                                        in_max=mk1[:], in_values=ka)
                    nc.vector.match_replace(out=ka, in_to_replace=mk1[:],
                                            in_values=ka, imm_value=-4.0)
                    mk2 = smallp.tile([128, 8], f32, tag=f"mk2{a}")
                    nc.vector.max(out=mk2[:], in_=ka)
                    nc.vector.max_index(out=ordb[:, 16 * a + 8 : 16 * a + 16],
                                        in_max=mk2[:], in_values=ka)
                    vcol = 2 + 10 * a
                    nc.scalar.activation(F[:, vcol : vcol + 8], mk1[:],
                                         AF.Copy, bias=0.0, scale=1.0)
                    nc.scalar.activation(F[:, vcol + 8 : vcol + 10], mk2[:, 0:2],
                                         AF.Copy, bias=0.0, scale=1.0)

                # companion fetch: gather cxy[idx] via the shared-stream
                # indirect_copy; x-sort comps = y coords (cxy cols 16+ord),
                # y-sort comps = x coords (cxy cols ord)
                idxg = smallp.tile([128, 20], u16, tag="idxg")
                nc.vector.tensor_scalar(out=idxg[:, 0:10], in0=ordb[:, 0:10],
                                        scalar1=16, scalar2=None, op0=OP.add)
                nc.vector.tensor_copy(out=idxg[:, 10:20], in_=ordb[:, 16:26])
                state[(b, t)] = (F, idxg, idx16, cxy, maskin)

            def stage_c(b, t):
                n0 = 128 * t
                rows = min(128, N - n0)
                F, idxg, idx16, cxy, maskin = state.pop((b, t))
                gath2 = smallp.tile([128, 320], f32, tag="gath2")
                nc.gpsimd.indirect_copy(
                    out=gath2[:],
                    data=cxy[:],
                    idxs=idxg[:],
                    i_know_ap_gather_is_preferred=True,
                )
                tmp2 = smallp.tile([128, 320], f32, tag="gtmp2")
                nc.vector.tensor_tensor(
                    out=tmp2[:].rearrange("p (m s) -> p m s", m=20, s=16),
                    in0=gath2[:].rearrange("p (m s) -> p m s", m=20, s=16),
                    in1=pmask[:].unsqueeze(1).to_broadcast([128, 20, 16]),
                    op=OP.mult)
                with nc.allow_low_precision("comp pick reduce; fp16 F"):
                    nc.vector.tensor_reduce(
                        out=F[:, 22:42].rearrange("p (a r) -> p a r", a=2, r=10),
                        in_=tmp2[:].rearrange("p (a r s) -> p a r s", a=2, r=10, s=16),
                        axis=mybir.AxisListType.X, op=OP.add)

                psum_t = psumTp.tile([NF, 128], f16, tag="ft")
                nc.tensor.transpose(psum_t[:], F[:], ident[:])
                fts = smallp.tile([NF, 128], f16, tag="fts")
                nc.scalar.activation(fts[:], psum_t[:], AF.Copy, bias=0.0, scale=1.0)
                psum_o = psumOp.tile([128, 128], f32, tag="o")
                nc.tensor.matmul(psum_o[:], fts[:], mt[:], start=True, stop=True)
                out_sb = smallp.tile([128, 128], f32, tag="outsb")
                nc.scalar.activation(out_sb[:], psum_o[:], AF.Copy, bias=0.0, scale=1.0)
                nc.sync.dma_start(y[b, n0 : n0 + rows, :], out_sb[0:rows, :])

                if debug:
                    nc.sync.dma_start(d_idx[b, t], idx16[:])
                    nc.sync.dma_start(d_cxy[b, t], cxy[:])
                    nc.sync.dma_start(d_mask[b, t, :, 0:16], maskin[:])
                    nc.sync.dma_start(d_f[b, t], F[:])

            chunks = [(b, t) for b in range(BL) for t in range(NCHUNK)]
            T = len(chunks)
            for k in range(T + 2):
                if 2 <= k:
                    stage_c(*chunks[k - 2])
                if k < T:
                    stage_a(*chunks[k])
                if 1 <= k < T + 1:
                    stage_b(*chunks[k - 1])

    if split:
        _split_multiwaits(nc, mybir)
    return nc


def _host_prep(x, Wx, bx, Wy, by, W1, b1, W2, b2):
    """Build per-core input maps."""
    import ml_dtypes

    bf16 = ml_dtypes.bfloat16

    x = np.asarray(x, dtype=np.float32)
    xc64 = x.astype(np.float64) - 0.5
    xc = xc64.astype(np.float32)
    r64 = xc64[..., 0] ** 2 + xc64[..., 1] ** 2

    xhi = xc.astype(bf16)
    xlo = (xc - xhi.astype(np.float32)).astype(bf16)
    rhi = r64.astype(np.float32).astype(bf16)
    rlo = (r64.astype(np.float32) - rhi.astype(np.float32)).astype(bf16)

    lhsd = np.zeros((B, 8, NPAD), bf16)
    lhsd[:, 0, :N] = (2.0 * xhi[..., 0].astype(np.float32)).astype(bf16)
    lhsd[:, 1, :N] = (2.0 * xhi[..., 1].astype(np.float32)).astype(bf16)
    lhsd[:, 2, :N] = lhsd[:, 0, :N]
    lhsd[:, 3, :N] = lhsd[:, 1, :N]
    lhsd[:, 4, :N] = (2.0 * xlo[..., 0].astype(np.float32)).astype(bf16)
    lhsd[:, 5, :N] = (2.0 * xlo[..., 1].astype(np.float32)).astype(bf16)
    lhsd[:, 6, :] = -1.0
    lhsd[:, 7, :] = -1.0

    rhsd = np.zeros((B, 8, NPAD), bf16)
    rhsd[:, 0, :N] = xhi[..., 0]
    rhsd[:, 1, :N] = xhi[..., 1]
    rhsd[:, 2, :N] = xlo[..., 0]
    rhsd[:, 3, :N] = xlo[..., 1]
    rhsd[:, 4, :N] = xhi[..., 0]
    rhsd[:, 5, :N] = xhi[..., 1]
    rhsd[:, 6, :N] = rhi
    rhsd[:, 6, N:] = 1.0e30
    rhsd[:, 7, :N] = rlo

    xypad = np.zeros((B, NPAD, 2), np.float32)
    xypad[:, :N] = x
    xytabd = np.broadcast_to(
        xypad.reshape(B, 1, 2 * NPAD), (B, 128, 2 * NPAD)
    ).copy()

    # aux: 0..7 = -r per chunk, 8..15 = -x, 16..23 = -y, 24..39 = (x,y)
    rpad = np.zeros((B, NPAD), np.float32)
    rpad[:, :N] = r64.astype(np.float32)
    rg = rpad.reshape(B, NCHUNK, 128)
    auxd = np.zeros((B, 128, 40), np.float32)
    xg = xypad.reshape(B, NCHUNK, 128, 2)
    for t in range(NCHUNK):
        auxd[:, :, t] = -rg[:, t]
        auxd[:, :, 8 + t] = -xg[:, t, :, 0]
        auxd[:, :, 16 + t] = -xg[:, t, :, 1]
        auxd[:, :, 24 + 2 * t] = xg[:, t, :, 0]
        auxd[:, :, 25 + 2 * t] = xg[:, t, :, 1]

    pmask = np.zeros((128, 16), np.float32)
    pmask[np.arange(128), np.arange(128) % 16] = 1.0
    ident = np.eye(128, dtype=np.float16)

    # fold all contractions into MT [43, H]; F layout:
    # [x, y, xvals 2..11, yvals 12..21, xcomps 22..31, ycomps 32..41, 1]
    W1_, W2_ = np.asarray(W1, np.float64), np.asarray(W2, np.float64)
    Wx_, Wy_ = np.asarray(Wx, np.float64), np.asarray(Wy, np.float64)
    bx_, by_ = np.asarray(bx, np.float64), np.asarray(by, np.float64)
    b1_, b2_ = np.asarray(b1, np.float64), np.asarray(b2, np.float64)
    mt = np.zeros((NF, H), np.float64)
    mt[0:2, :] = W1_
    # the kernel sorts DESCENDING: F slot j holds ascending rank 9-j
    for k in range(K):
        mt[2 + k, :] = Wx_[:, 0, 9 - k] @ W2_   # sorted-by-x x-coords
        mt[12 + k, :] = Wy_[:, 1, 9 - k] @ W2_  # sorted-by-y y-coords
        mt[22 + k, :] = Wx_[:, 1, 9 - k] @ W2_  # sorted-by-x y-companions
        mt[32 + k, :] = Wy_[:, 0, 9 - k] @ W2_  # sorted-by-y x-companions
    mt[42, :] = b1_ + b2_ + (bx_ + by_) @ W2_
    mt = mt.astype(np.float16)

    in_maps = []
    for core in range(NCORES):
        sl = slice(core * BL, (core + 1) * BL)
        in_maps.append({
            "lhsd": lhsd[sl], "rhsd": rhsd[sl], "xytabd": xytabd[sl],
            "auxd": auxd[sl],
            "pmask": pmask, "ident": ident, "mt": mt,
        })
    return in_maps


_CACHE = {}


def _get_program(debug=False):
    key = bool(debug)
    if key not in _CACHE:
        _CACHE[key] = _build_program(debug=debug)
    return _CACHE[key]


def kernel(x, Wx, bx, Wy, by, W1, b1, W2, b2, _debug=False, _trace=False):
    from concourse.bass_utils import run_bass_kernel_spmd

    nc = _get_program(debug=_debug)
    in_maps = _host_prep(x, Wx, bx, Wy, by, W1, b1, W2, b2)
    res = run_bass_kernel_spmd(nc, in_maps, list(range(NCORES)), trace=_trace)
    out = np.concatenate([res.results[i]["y"] for i in range(NCORES)], axis=0)
    if _debug or _trace:
        kernel._last = res
    return out


# revision 25
# speedup vs baseline: 1.9045x; 1.9045x over previous
"""Trainium2 Bass kernel for nn_ConvEmbeddingXY (retrieval_knn).

Problem: B=32 batches of N=1000 2-D points. Per point: node embedding
(x @ W1 + b1), 10-NN by squared distance (incl. self), neighbor coords
sorted by x and by y feed two tiny convs, conv outputs go through W2 and
sum with the node embedding.

Strategy (data-parallel over B across 8 cores, 4 batches/core), v2:
  - distances via a split-precision bf16 PE matmul (x = xhi + xlo in bf16;
    u = 2x_i.x_j - r_j up to ~1e-5 abs error; the per-row -r_i shift is
    dropped entirely since per-row ranking is shift-invariant)
  - exact top-16 candidates via DVE max8/max_index/match_replace on the
    f32 psum copy of u
  - candidate (x,y) pairs via GPSIMD indirect_copy (16-partition-group
    shared index stream) + masked pick; the pick multiply runs on GPSIMD,
    the pick reduce on DVE
  - refine: d2 recomputed exactly like the reference from gathered f32
    coords; top-10 marked via a +1e30-sentinel match_replace round and a
    per-partition threshold compare against the 10th value
  - per-axis sort of the 10 pairs via max8 on negated coords; companions
    via a one-hot (GPSIMD is_equal+multiply) and one fused DVE reduce
  - output: fp16 F[43] features transposed on PE, one fp16 matmul with the
    host-folded [43, H] weight matrix
  - the per-chunk work is software-pipelined at depth 3 (A: matmul+scan+
    gather, B: pick+refine+sort, C: companions+output) so the DVE never
    waits on GPSIMD round-trips.
"""

import numpy as np

B, N, K, H, C = 32, 1000, 10, 128, 2
NPAD = 1024
NCORES = 8
BL = B // NCORES          # batches per core
NCHUNK = NPAD // 128      # 128-point chunks per batch
NF = 43                   # x,y | xvals 10 | yvals 10 | xcomps 10 | ycomps 10 | 1

_SENT = 1.0e30            # refine round-1 sentinel (positive!)


def _split_multiwaits(nc, mybir):
    """This container's walrus build accepts at most ONE sync-wait command per
    instruction. Tile attaches several; redistribute extras onto same-engine
    NoOp carriers placed immediately before the instruction."""
    counter = 0
    for fn in nc.m.functions:
        for blk in fn.blocks:
            insts = blk.instructions
            new = []
            changed = False
            for inst in insts:
                si = inst.sync_info
                waits = list(si.on_wait) if (si is not None and si.on_wait) else []
                if len(waits) > 1:
                    for w in waits[:-1]:
                        counter += 1
                        nop = mybir.InstNoOp(
                            name=f"I-waitcarrier-{counter}", ins=[], outs=[]
                        )
                        nop.engine = inst.engine
                        nop.sync_info = mybir.SyncInfo(on_wait=[w], on_update=[])
                        new.append(nop)
                    inst.sync_info = mybir.SyncInfo(
                        on_wait=[waits[-1]],
                        on_update=list(si.on_update) if si.on_update else [],
                    )
                    changed = True
                new.append(inst)
            if changed:
                blk.instructions = new


def _build_program(debug=False, split=True):
    import concourse.bass as bass
    import concourse.mybir as mybir
    from concourse.tile import TileContext

    f32 = mybir.dt.float32
    f16 = mybir.dt.float16
    bf16 = mybir.dt.bfloat16
    u16 = mybir.dt.uint16
    u8 = mybir.dt.uint8
    AF = mybir.ActivationFunctionType
    OP = mybir.AluOpType

    nc = bass.Bass()

    lhsd = nc.dram_tensor("lhsd", [BL, 8, NPAD], bf16, kind="ExternalInput")
    rhsd = nc.dram_tensor("rhsd", [BL, 8, NPAD], bf16, kind="ExternalInput")
    xytabd = nc.dram_tensor("xytabd", [BL, 128, 2 * NPAD], f32, kind="ExternalInput")
    auxd = nc.dram_tensor("auxd", [BL, 128, 40], f32, kind="ExternalInput")
    pmask_d = nc.dram_tensor("pmask", [128, 16], f32, kind="ExternalInput")
    iota16_d = nc.dram_tensor("iota16", [128, 16], f32, kind="ExternalInput")
    ident_d = nc.dram_tensor("ident", [128, 128], f16, kind="ExternalInput")
    mt_d = nc.dram_tensor("mt", [NF, 128], f16, kind="ExternalInput")

    y = nc.dram_tensor("y", [BL, N, H], f32, kind="ExternalOutput")
    if debug:
        d_idx = nc.dram_tensor("d_idx", [BL, NCHUNK, 128, 16], u16, kind="ExternalOutput")
        d_cxy = nc.dram_tensor("d_cxy", [BL, NCHUNK, 128, 32], f32, kind="ExternalOutput")
        d_mask = nc.dram_tensor("d_mask", [BL, NCHUNK, 128, 32], u8, kind="ExternalOutput")
        d_f = nc.dram_tensor("d_f", [BL, NCHUNK, 128, NF], f16, kind="ExternalOutput")

    with TileContext(nc) as tc:
        with (
            tc.tile_pool(name="const", bufs=1) as constp,
            tc.tile_pool(name="batch", bufs=2) as batchp,
            tc.tile_pool(name="big", bufs=3) as bigp,
            tc.tile_pool(name="small", bufs=4) as smallp,
            tc.tile_pool(name="psumG", bufs=2, space="PSUM") as psumGp,
            tc.tile_pool(name="psumT", bufs=2, space="PSUM") as psumTp,
            tc.tile_pool(name="psumO", bufs=2, space="PSUM") as psumOp,
        ):
            pmask = constp.tile([128, 16], f32)
            iota16 = constp.tile([128, 16], f32)
            ident = constp.tile([128, 128], f16)
            mt = constp.tile([NF, 128], f16)
            nc.sync.dma_start(pmask[:], pmask_d[:])
            nc.sync.dma_start(iota16[:], iota16_d[:])
            nc.sync.dma_start(ident[:], ident_d[:])
            nc.sync.dma_start(mt[:], mt_d[:])

            batch_tiles = {}

            def load_batch(b):
                lhs_sb = batchp.tile([8, NPAD], bf16, tag="lhs")
                rhs_sb = batchp.tile([8, NPAD], bf16, tag="rhs")
                xytab = batchp.tile([128, 2 * NPAD], f32, tag="xytab")
                aux = batchp.tile([128, 40], f32, tag="aux")
                nc.sync.dma_start(lhs_sb[:], lhsd[b])
                nc.sync.dma_start(rhs_sb[:], rhsd[b])
                nc.sync.dma_start(xytab[:], xytabd[b])
                nc.sync.dma_start(aux[:], auxd[b])
                batch_tiles[b] = (lhs_sb, rhs_sb, xytab, aux)

            state = {}

            def stage_a(b, t):
                if t == 0 and b not in batch_tiles:
                    load_batch(b)
                lhs_sb, rhs_sb, xytab, aux = batch_tiles[b]
                n0 = 128 * t

                psum_g = psumGp.tile([128, N], f32, tag="g")
                lhsT = lhs_sb[:, n0 : n0 + 128]
                nc.tensor.matmul(psum_g[:, 0:512], lhsT, rhs_sb[:, 0:512],
                                 start=True, stop=True)
                nc.tensor.matmul(psum_g[:, 512:N], lhsT, rhs_sb[:, 512:N],
                                 start=True, stop=True)

                negr = aux[:, t : t + 1]
                u = bigp.tile([128, N], f32, tag="u")
                nc.scalar.activation(u[:, 0:512], psum_g[:, 0:512], AF.Identity,
                                     bias=negr, scale=1.0)
                nc.scalar.activation(u[:, 512:N], psum_g[:, 512:N], AF.Identity,
                                     bias=negr, scale=1.0)

                m8a = smallp.tile([128, 8], f32, tag="m8a")
                m8b = smallp.tile([128, 8], f32, tag="m8b")
                idx16 = smallp.tile([128, 16], u16, tag="idx16")
                idx2 = smallp.tile([128, 16], u16, tag="idx2")
                gath = bigp.tile([128, 512], f32, tag="gath")
                data3 = xytab[:].rearrange("p (n c) -> p n c", n=NPAD, c=2)
                nc.vector.max(out=m8a[:], in_=u[:])
                nc.vector.max_index(out=idx16[:, 0:8], in_max=m8a[:], in_values=u[:])
                nc.vector.tensor_scalar(out=idx2[:, 0:8], in0=idx16[:, 0:8],
                                        scalar1=2, scalar2=None, op0=OP.mult)
                nc.gpsimd.indirect_copy(
                    out=gath[:, 0:256].rearrange("p (i c) -> p i c", i=128, c=2),
                    data=data3, idxs=idx2[:, 0:8],
                    i_know_ap_gather_is_preferred=True,
                )
                nc.vector.match_replace(out=u[:], in_to_replace=m8a[:],
                                        in_values=u[:], imm_value=-1.0e30)
                nc.vector.max(out=m8b[:], in_=u[:])
                nc.vector.max_index(out=idx16[:, 8:16], in_max=m8b[:], in_values=u[:])
                nc.vector.tensor_scalar(out=idx2[:, 8:16], in0=idx16[:, 8:16],
                                        scalar1=2, scalar2=None, op0=OP.mult)
                nc.gpsimd.indirect_copy(
                    out=gath[:, 256:512].rearrange("p (i c) -> p i c", i=128, c=2),
                    data=data3, idxs=idx2[:, 8:16],
                    i_know_ap_gather_is_preferred=True,
                )
                state[(b, t)] = (gath, idx16)

            def stage_b(b, t):
                _, _, _, aux = batch_tiles[b]
                gath, idx16 = state[(b, t)]

                tmp = bigp.tile([128, 512], f32, tag="gtmp")
                gv = gath[:].rearrange("p (m s c) -> p m s c", m=16, s=16, c=2)
                pm = pmask[:].unsqueeze(1).unsqueeze(3).to_broadcast([128, 16, 16, 2])
                nc.vector.tensor_tensor(
                    out=tmp[:].rearrange("p (m s c) -> p m s c", m=16, s=16, c=2),
                    in0=gv, in1=pm, op=OP.mult)
                # pick: cxy[p, 16c+m] = sum_s tmp[p, m, s, c]
                cxy = smallp.tile([128, 32], f32, tag="cxy")
                nc.vector.tensor_reduce(
                    out=cxy[:].rearrange("p (c m) -> p m c", c=2, m=16),
                    in_=tmp[:].rearrange("p (m s c) -> p m s c", m=16, s=16, c=2)
                        .transpose([0, 1, 3, 2]),
                    axis=mybir.AxisListType.X, op=OP.add)
                cx16 = cxy[:, 0:16]
                cy16 = cxy[:, 16:32]

                # refine: d2 recomputed exactly as in the reference
                negx = aux[:, 8 + t : 9 + t]
                negy = aux[:, 16 + t : 17 + t]
                dx2 = smallp.tile([128, 16], f32, tag="dx2")
                dy2 = smallp.tile([128, 16], f32, tag="dy2")
                nc.scalar.activation(dx2[:], cx16, AF.Square, bias=negx, scale=1.0)
                nc.scalar.activation(dy2[:], cy16, AF.Square, bias=negy, scale=1.0)
                s16 = smallp.tile([128, 16], f32, tag="s16")
                nc.vector.scalar_tensor_tensor(out=s16[:], in0=dx2[:], scalar=-1.0,
                                               in1=dy2[:], op0=OP.mult,
                                               op1=OP.subtract)
                # find v10 = 10th-largest of s16 on a scratch copy
                s16b = smallp.tile([128, 16], f32, tag="s16b")
                nc.vector.tensor_copy(out=s16b[:], in_=s16[:])
                mc1 = smallp.tile([128, 8], f32, tag="mc1")
                nc.vector.max(out=mc1[:], in_=s16b[:])
                nc.vector.match_replace(out=s16b[:], in_to_replace=mc1[:],
                                        in_values=s16b[:], imm_value=-_SENT)
                mc2 = smallp.tile([128, 8], f32, tag="mc2")
                nc.vector.max(out=mc2[:], in_=s16b[:])
                # included = s16 >= v10 (exactly the top-10 slots, ties aside)
                maskin = smallp.tile([128, 16], u8, tag="maskin")
                nc.vector.tensor_scalar(out=maskin[:], in0=s16[:],
                                        scalar1=mc2[:, 1:2], scalar2=None,
                                        op0=OP.is_ge)

                # keys: kx = -4 default, included slots -> +coord (sort DESC;
                # the host reverses the MT rows to undo the ordering)
                kx = smallp.tile([128, 32], f32, tag="kx")
                nc.vector.memset(kx[:], -4.0)
                nc.vector.copy_predicated(
                    out=kx[:].rearrange("p (a j) -> p a j", a=2, j=16),
                    mask=maskin[:].unsqueeze(1).to_broadcast([128, 2, 16]),
                    data=cxy[:].rearrange("p (a j) -> p a j", a=2, j=16))

                F = smallp.tile([128, NF], f16, tag="F")
                nc.scalar.activation(F[:, 0:2], aux[:, 24 + 2 * t : 26 + 2 * t],
                                     AF.Copy, bias=0.0, scale=1.0)
                nc.vector.memset(F[:, 42:43], 1.0)

                ordb = smallp.tile([128, 32], u16, tag="ordb")
                for a in (0, 1):
                    ka = kx[:, 16 * a : 16 * a + 16]
                    mk1 = smallp.tile([128, 8], f32, tag=f"mk1{a}")
                    nc.vector.max(out=mk1[:], in_=ka)
                    nc.vector.max_index(out=ordb[:, 16 * a : 16 * a + 8],
                                        in_max=mk1[:], in_values=ka)
                    nc.vector.match_replace(out=ka, in_to_replace=mk1[:],
                                            in_values=ka, imm_value=-4.0)
                    mk2 = smallp.tile([128, 8], f32, tag=f"mk2{a}")
                    nc.vector.max(out=mk2[:], in_=ka)
                    nc.vector.max_index(out=ordb[:, 16 * a + 8 : 16 * a + 16],
                                        in_max=mk2[:], in_values=ka)
                    vcol = 2 + 10 * a
                    nc.scalar.activation(F[:, vcol : vcol + 8], mk1[:],
                                         AF.Copy, bias=0.0, scale=1.0)
                    nc.scalar.activation(F[:, vcol + 8 : vcol + 10], mk2[:, 0:2],
                                         AF.Copy, bias=0.0, scale=1.0)

                # companion fetch: gather cxy[idx] via the shared-stream
                # indirect_copy; x-sort comps = y coords (cxy cols 16+ord),
                # y-sort comps = x coords (cxy cols ord)
                ordf = smallp.tile([128, 20], f32, tag="ordf")
                nc.vector.tensor_copy(
                    out=ordf[:].rearrange("p (a r) -> p a r", a=2, r=10),
                    in_=ordb[:].rearrange("p (a j) -> p a j", a=2, j=16)[:, :, 0:10])
                oh = smallp.tile([128, 320], f32, tag="oh")
                nc.vector.tensor_tensor(
                    out=oh[:].rearrange("p (r j) -> p r j", r=20, j=16),
                    in0=ordf[:].unsqueeze(2).to_broadcast([128, 20, 16]),
                    in1=iota16[:].unsqueeze(1).to_broadcast([128, 20, 16]),
                    op=OP.is_equal)
                ohm = smallp.tile([128, 320], f32, tag="ohm")
                # x-sort companions are y coords; y-sort companions are x coords
                nc.vector.tensor_tensor(
                    out=ohm[:, 0:160].rearrange("p (r j) -> p r j", r=10, j=16),
                    in0=oh[:, 0:160].rearrange("p (r j) -> p r j", r=10, j=16),
                    in1=cy16.unsqueeze(1).to_broadcast([128, 10, 16]), op=OP.mult)
                nc.vector.tensor_tensor(
                    out=ohm[:, 160:320].rearrange("p (r j) -> p r j", r=10, j=16),
                    in0=oh[:, 160:320].rearrange("p (r j) -> p r j", r=10, j=16),
                    in1=cx16.unsqueeze(1).to_broadcast([128, 10, 16]), op=OP.mult)
                state[(b, t)] = (F, ohm, idx16, cxy, maskin)

            def stage_c(b, t):
                n0 = 128 * t
                rows = min(128, N - n0)
                F, ohm, idx16, cxy, maskin = state.pop((b, t))
                with nc.allow_low_precision("one-hot comp reduce; fp16 F"):
                    nc.vector.tensor_reduce(
                        out=F[:, 22:42].rearrange("p (a r) -> p a r", a=2, r=10),
                        in_=ohm[:].rearrange("p (a r j) -> p a r j", a=2, r=10, j=16),
                        axis=mybir.AxisListType.X, op=OP.add)

                psum_t = psumTp.tile([NF, 128], f16, tag="ft")
                nc.tensor.transpose(psum_t[:], F[:], ident[:])
                fts = smallp.tile([NF, 128], f16, tag="fts")
                nc.scalar.activation(fts[:], psum_t[:], AF.Copy, bias=0.0, scale=1.0)
                psum_o = psumOp.tile([128, 128], f32, tag="o")
                nc.tensor.matmul(psum_o[:], fts[:], mt[:], start=True, stop=True)
                out_sb = smallp.tile([128, 128], f32, tag="outsb")
                nc.scalar.activation(out_sb[:], psum_o[:], AF.Copy, bias=0.0, scale=1.0)
                nc.sync.dma_start(y[b, n0 : n0 + rows, :], out_sb[0:rows, :])

                if debug:
                    nc.sync.dma_start(d_idx[b, t], idx16[:])
                    nc.sync.dma_start(d_cxy[b, t], cxy[:])
                    nc.sync.dma_start(d_mask[b, t, :, 0:16], maskin[:])
                    nc.sync.dma_start(d_f[b, t], F[:])

            chunks = [(b, t) for b in range(BL) for t in range(NCHUNK)]
            T = len(chunks)
            for k in range(T + 2):
                if 2 <= k:
                    stage_c(*chunks[k - 2])
                if k < T:
                    stage_a(*chunks[k])
                if 1 <= k < T + 1:
                    stage_b(*chunks[k - 1])

    if split:
        _split_multiwaits(nc, mybir)
    return nc


def _host_prep(x, Wx, bx, Wy, by, W1, b1, W2, b2):
    """Build per-core input maps."""
    import ml_dtypes

    bf16 = ml_dtypes.bfloat16

    x = np.asarray(x, dtype=np.float32)
    xc64 = x.astype(np.float64) - 0.5
    xc = xc64.astype(np.float32)
    r64 = xc64[..., 0] ** 2 + xc64[..., 1] ** 2

    xhi = xc.astype(bf16)
    xlo = (xc - xhi.astype(np.float32)).astype(bf16)
    rhi = r64.astype(np.float32).astype(bf16)
    rlo = (r64.astype(np.float32) - rhi.astype(np.float32)).astype(bf16)

    lhsd = np.zeros((B, 8, NPAD), bf16)
    lhsd[:, 0, :N] = (2.0 * xhi[..., 0].astype(np.float32)).astype(bf16)
    lhsd[:, 1, :N] = (2.0 * xhi[..., 1].astype(np.float32)).astype(bf16)
    lhsd[:, 2, :N] = lhsd[:, 0, :N]
    lhsd[:, 3, :N] = lhsd[:, 1, :N]
    lhsd[:, 4, :N] = (2.0 * xlo[..., 0].astype(np.float32)).astype(bf16)
    lhsd[:, 5, :N] = (2.0 * xlo[..., 1].astype(np.float32)).astype(bf16)
    lhsd[:, 6, :] = -1.0
    lhsd[:, 7, :] = -1.0

    rhsd = np.zeros((B, 8, NPAD), bf16)
    rhsd[:, 0, :N] = xhi[..., 0]
    rhsd[:, 1, :N] = xhi[..., 1]
    rhsd[:, 2, :N] = xlo[..., 0]
    rhsd[:, 3, :N] = xlo[..., 1]
    rhsd[:, 4, :N] = xhi[..., 0]
    rhsd[:, 5, :N] = xhi[..., 1]
    rhsd[:, 6, :N] = rhi
    rhsd[:, 6, N:] = 1.0e30
    rhsd[:, 7, :N] = rlo

    xypad = np.zeros((B, NPAD, 2), np.float32)
    xypad[:, :N] = x
    xytabd = np.broadcast_to(
        xypad.reshape(B, 1, 2 * NPAD), (B, 128, 2 * NPAD)
    ).copy()

    # aux: 0..7 = -r per chunk, 8..15 = -x, 16..23 = -y, 24..39 = (x,y)
    rpad = np.zeros((B, NPAD), np.float32)
    rpad[:, :N] = r64.astype(np.float32)
    rg = rpad.reshape(B, NCHUNK, 128)
    auxd = np.zeros((B, 128, 40), np.float32)
    xg = xypad.reshape(B, NCHUNK, 128, 2)
    for t in range(NCHUNK):
        auxd[:, :, t] = -rg[:, t]
        auxd[:, :, 8 + t] = -xg[:, t, :, 0]
        auxd[:, :, 16 + t] = -xg[:, t, :, 1]
        auxd[:, :, 24 + 2 * t] = xg[:, t, :, 0]
        auxd[:, :, 25 + 2 * t] = xg[:, t, :, 1]

    pmask = np.zeros((128, 16), np.float32)
    pmask[np.arange(128), np.arange(128) % 16] = 1.0
    iota16 = np.tile(np.arange(16, dtype=np.float32), (128, 1))
    ident = np.eye(128, dtype=np.float16)

    # fold all contractions into MT [43, H]; F layout:
    # [x, y, xvals 2..11, yvals 12..21, xcomps 22..31, ycomps 32..41, 1]
    W1_, W2_ = np.asarray(W1, np.float64), np.asarray(W2, np.float64)
    Wx_, Wy_ = np.asarray(Wx, np.float64), np.asarray(Wy, np.float64)
    bx_, by_ = np.asarray(bx, np.float64), np.asarray(by, np.float64)
    b1_, b2_ = np.asarray(b1, np.float64), np.asarray(b2, np.float64)
    mt = np.zeros((NF, H), np.float64)
    mt[0:2, :] = W1_
    # the kernel sorts DESCENDING: F slot j holds ascending rank 9-j
    for k in range(K):
        mt[2 + k, :] = Wx_[:, 0, 9 - k] @ W2_   # sorted-by-x x-coords
        mt[12 + k, :] = Wy_[:, 1, 9 - k] @ W2_  # sorted-by-y y-coords
        mt[22 + k, :] = Wx_[:, 1, 9 - k] @ W2_  # sorted-by-x y-companions
        mt[32 + k, :] = Wy_[:, 0, 9 - k] @ W2_  # sorted-by-y x-companions
    mt[42, :] = b1_ + b2_ + (bx_ + by_) @ W2_
    mt = mt.astype(np.float16)

    in_maps = []
    for core in range(NCORES):
        sl = slice(core * BL, (core + 1) * BL)
        in_maps.append({
            "lhsd": lhsd[sl], "rhsd": rhsd[sl], "xytabd": xytabd[sl],
            "auxd": auxd[sl],
            "pmask": pmask, "iota16": iota16, "ident": ident, "mt": mt,
        })
    return in_maps


_CACHE = {}


def _get_program(debug=False):
    key = bool(debug)
    if key not in _CACHE:
        _CACHE[key] = _build_program(debug=debug)
    return _CACHE[key]


def kernel(x, Wx, bx, Wy, by, W1, b1, W2, b2, _debug=False, _trace=False):
    from concourse.bass_utils import run_bass_kernel_spmd

    nc = _get_program(debug=_debug)
    in_maps = _host_prep(x, Wx, bx, Wy, by, W1, b1, W2, b2)
    res = run_bass_kernel_spmd(nc, in_maps, list(range(NCORES)), trace=_trace)
    out = np.concatenate([res.results[i]["y"] for i in range(NCORES)], axis=0)
    if _debug or _trace:
        kernel._last = res
    return out
